# revision 1
# baseline (speedup 1.0000x reference)
"""Trainium2 Bass kernel for nn_MultiHeadAttn (B=4, NQ=NK=2048, D=1024, H=8).

Sharding: 8 cores = 4 batches x 2 query-halves. Each core owns 1024 query rows
of one batch; k/v projections for that batch are computed redundantly by the
two cores sharing it (cheaper than collectives for this size).

Per-core dataflow (all activations feature-major "T layout" [feat, row]):
  qpT = Wq @ qT          (f32r ~ TF32)
  kpT = (Wk/32) @ kT     (bf16)
  vp  = v @ Wv.T         (bf16, natural [key, feat] layout)
  per head, per 512-row chunk, flash-style over 16 key tiles:
      logitsT[kk,r] = kpT_h_tile.T @ qpT_h        (bf16 matmul, PSUM f32)
      expT = Exp(logitsT + mask_bias[kk])         (ACT, per-partition bias)
      attT += vp_tile.T @ expT                     (PSUM accumulate)
      den  += ones.T @ expT                        (PSUM accumulate)
  x1T = qpT + attT / den
  out1 = LN(x1) via ones-matmul stats (sums over feature partitions)
  x2T = out1 + Relu(Wout @ out1T + bout)           (f32r matmul, ACT bias+relu)
  outT = LN(x2)  -> DRAM [feat, row]; host transposes back.
"""

from contextlib import ExitStack

import numpy as np
import ml_dtypes

import concourse.mybir as mybir
import concourse.tile as tile
from concourse import bacc
from concourse.bass_utils import run_bass_kernel_spmd

B, NQ, NK, D, H = 4, 2048, 2048, 1024, 8
DH = D // H            # 128, head dim
P = 128                # partitions
RQ = NQ // 2           # 1024 query rows per core
EPS = 1e-5

F32 = mybir.dt.float32
F32R = mybir.dt.float32r
BF16 = mybir.dt.bfloat16
BFNP = ml_dtypes.bfloat16

KT = D // P            # 8 contraction tiles over features
DT = D // P            # 8 output-feature tiles (also heads)
KKT = NK // P          # 16 key tiles
RC = RQ // 512         # 2 row chunks of 512


def build_nc():
    nc = bacc.Bacc("TRN2", target_bir_lowering=False)

    qT = nc.declare_dram_parameter("qT", [D, RQ], F32R, isOutput=False)
    kT = nc.declare_dram_parameter("kT", [D, NK], BF16, isOutput=False)
    vT = nc.declare_dram_parameter("vT", [D, NK], BF16, isOutput=False)
    wqT = nc.declare_dram_parameter("wqT", [D, D], F32R, isOutput=False)
    wkT = nc.declare_dram_parameter("wkT", [D, D], BF16, isOutput=False)
    wvT = nc.declare_dram_parameter("wvT", [D, D], BF16, isOutput=False)
    woT = nc.declare_dram_parameter("woT", [D, D], F32R, isOutput=False)
    maskb = nc.declare_dram_parameter("maskb", [P, KKT], F32, isOutput=False)
    g1 = nc.declare_dram_parameter("g1", [P, DT], F32, isOutput=False)
    b1 = nc.declare_dram_parameter("b1", [P, DT], F32, isOutput=False)
    g2 = nc.declare_dram_parameter("g2", [P, DT], F32, isOutput=False)
    b2 = nc.declare_dram_parameter("b2", [P, DT], F32, isOutput=False)
    bo = nc.declare_dram_parameter("bo", [P, DT], F32, isOutput=False)
    outT = nc.declare_dram_parameter("outT", [D, RQ], F32, isOutput=True)

    Act = mybir.ActivationFunctionType

    with tile.TileContext(nc) as tc, ExitStack() as ctx:
        consts = ctx.enter_context(tc.tile_pool(name="consts", bufs=1))
        pool_qp = ctx.enter_context(tc.tile_pool(name="pool_qp", bufs=1))

        ones_bf = consts.tile([P, P], BF16)
        nc.vector.memset(ones_bf, 1.0)
        onesn = consts.tile([P, P], BF16)
        nc.vector.memset(onesn, 1.0 / D)
        eps_sb = consts.tile([P, 1], F32)
        nc.vector.memset(eps_sb, EPS)
        maskb_sb = consts.tile([P, KKT], F32)
        nc.sync.dma_start(out=maskb_sb, in_=maskb[:, :])
        # maskones[:, kkt, :]: column kkt's 0/1 mask replicated 32 wide (bf16)
        maskones = consts.tile([P, KKT, 32], BF16)
        for t in range(KKT):
            nc.vector.tensor_scalar_mul(
                maskones[:, t, :], ones_bf[:, 0:32], maskb_sb[:, t:t + 1]
            )
        # f32r all-(1/32) for summing the 4 stacked den blocks + broadcast
        ones32r = consts.tile([P, P], F32R)
        nc.scalar.mul(ones32r, ones_bf, 1.0 / 32.0)
        g1_sb = consts.tile([P, DT], F32)
        nc.sync.dma_start(out=g1_sb, in_=g1[:, :])
        b1_sb = consts.tile([P, DT], F32)
        nc.sync.dma_start(out=b1_sb, in_=b1[:, :])
        g2_sb = consts.tile([P, DT], F32)
        nc.sync.dma_start(out=g2_sb, in_=g2[:, :])
        b2_sb = consts.tile([P, DT], F32)
        nc.sync.dma_start(out=b2_sb, in_=b2[:, :])
        bo_sb = consts.tile([P, DT], F32)
        nc.sync.dma_start(out=bo_sb, in_=bo[:, :])

        # persistent activations
        qpT_sb = pool_qp.tile([P, DT, RQ], F32)      # qp.T; becomes x1T then x2T
        xbf_sb = pool_qp.tile([P, DT, RQ], BF16)     # bf16 shadow (qp, then x1, x2)

        with (
            tc.tile_pool(name="pool_attn", bufs=1) as pool_attn,
            tc.tile_pool(name="pool_ain", bufs=1) as ain,
        ):
            kpT_sb = pool_attn.tile([P, H, NK], BF16)    # per-head [dh, key]
            vp_sb = pool_attn.tile([P, KKT, D], BF16)    # per key-tile [key, feat]
            # ------------- Phase A: q and v projections ----------
            # k is projected per-head inside the attention section so PE work
            # interleaves with the ACT-bound exp stream.
            # DMA slots: "qk" holds qT then kT; "vv" holds vT; "w" rotates
            # wqA -> wqB -> wv -> wk (16KB each, weight halves for q).
            with tc.tile_pool(name="a_ps", bufs=3, space="PSUM") as a_ps:
                wqA_sb = ain.tile([P, KT, 512], F32R, tag="w")
                qT_sb = ain.tile([P, KT, RQ], F32R, tag="qk")
                for t in range(KT):
                    nc.sync.dma_start(out=wqA_sb[:, t, :], in_=wqT[t * P:(t + 1) * P, 0:512])
                    nc.sync.dma_start(out=qT_sb[:, t, 0:512], in_=qT[t * P:(t + 1) * P, 0:512])
                    nc.sync.dma_start(out=qT_sb[:, t, 512:1024], in_=qT[t * P:(t + 1) * P, 512:1024])
                wqB_sb = ain.tile([P, KT, 512], F32R, tag="w")
                for t in range(KT):
                    nc.sync.dma_start(out=wqB_sb[:, t, :], in_=wqT[t * P:(t + 1) * P, 512:1024])
                vT_sb = ain.tile([P, KT, NK], BF16, tag="vv")
                for t in range(KT):
                    nc.sync.dma_start(out=vT_sb[:, t, :], in_=vT[t * P:(t + 1) * P, :])

                def q_proj(w_sb, dt0):
                    for dt_ in range(dt0, dt0 + 4):
                        for c in range(RC):
                            ps = a_ps.tile([P, 512], F32, tag="aps")
                            for kt in range(KT):
                                nc.tensor.matmul(
                                    ps,
                                    w_sb[:, kt, (dt_ - dt0) * P:(dt_ - dt0 + 1) * P],
                                    qT_sb[:, kt, c * 512:(c + 1) * 512],
                                    start=(kt == 0), stop=(kt == KT - 1),
                                )
                            nc.vector.tensor_copy(qpT_sb[:, dt_, c * 512:(c + 1) * 512], ps)
                            nc.vector.tensor_copy(xbf_sb[:, dt_, c * 512:(c + 1) * 512], ps)

                q_proj(wqA_sb, 0)
                q_proj(wqB_sb, 4)

                # v projection (bf16, natural layout): vp[kk, dout] = v @ Wv.T
                # masked key rows are zeroed at drain time (mask01 per-partition)
                wvT_sb = ain.tile([P, KT, D], BF16, tag="w")
                for t in range(KT):
                    nc.sync.dma_start(out=wvT_sb[:, t, :], in_=wvT[t * P:(t + 1) * P, :])
                kT_sb = ain.tile([P, KT, NK], BF16, tag="qk")
                for t in range(KT):
                    nc.sync.dma_start(out=kT_sb[:, t, :], in_=kT[t * P:(t + 1) * P, :])
                for kkt in range(KKT):
                    for c in range(D // 512):
                        ps = a_ps.tile([P, 512], F32, tag="aps")
                        for kt in range(KT):
                            nc.tensor.matmul(
                                ps,
                                vT_sb[:, kt, kkt * P:(kkt + 1) * P],
                                wvT_sb[:, kt, c * 512:(c + 1) * 512],
                                start=(kt == 0), stop=(kt == KT - 1),
                            )
                        nc.vector.tensor_scalar_mul(
                            vp_sb[:, kkt, c * 512:(c + 1) * 512], ps,
                            maskb_sb[:, kkt:kkt + 1],
                        )

            wkT_sb = ain.tile([P, KT, D], BF16, tag="w")
            for t in range(KT):
                nc.sync.dma_start(out=wkT_sb[:, t, :], in_=wkT[t * P:(t + 1) * P, :])

            # ------------- Phase B: k projection + attention, per head -------
            # Mask-free exp (masked keys excluded via zeroed vp rows and masked
            # den lhsT). Emission is software-pipelined one super-group (4 key
            # tiles) ahead, and each (h,c) iteration's drain chain (den sum ->
            # reciprocal -> normalize -> residual) is deferred into the next
            # iteration so the in-order PE stream never waits on ACT/DVE.
            with (
                tc.tile_pool(name="att_ps", bufs=1, space="PSUM") as att_psp,
                tc.tile_pool(name="den_ps", bufs=1, space="PSUM") as den_psp,
                tc.tile_pool(name="lg_ps", bufs=2, space="PSUM") as lg_psp,
                tc.tile_pool(name="kp_ps", bufs=2, space="PSUM") as kp_psp,
                tc.tile_pool(name="bsb", bufs=1) as bsb,
            ):
                NSG = KKT // 4  # 4 super-groups of 4 key tiles (2 exp pairs)
                pending = []    # deferred drain state: (h, rs, att_ps, den_ps)

                def emit_drain():
                    if not pending:
                        return
                    h, rs, att_ps, den_ps = pending.pop()
                    # den blocks -> f32r SBUF -> (1/32)-matmul sum + broadcast
                    dsb = bsb.tile([P, 512], F32, tag="dsb", bufs=1)
                    nc.vector.tensor_copy(dsb[:, :].bitcast(F32R), den_ps)
                    dbc = den_psp.tile([P, 512], F32, tag="den")
                    nc.tensor.matmul(
                        dbc, ones32r, dsb[:, :].bitcast(F32R),
                        start=True, stop=True,
                    )
                    rec = bsb.tile([P, 512], F32, tag="rec", bufs=1)
                    nc.vector.reciprocal_approx_fast(rec, dbc)
                    nc.vector.tensor_mul(rec, att_ps, rec)  # in-place att/den
                    # x1 = qp + att  (in place over qpT)
                    nc.vector.tensor_add(qpT_sb[:, h, rs], qpT_sb[:, h, rs], rec)
                    nc.vector.tensor_copy(xbf_sb[:, h, rs], qpT_sb[:, h, rs])

                for h in range(H):
                    # k projection for this head: kpT[h, :] = (Wk/32) @ k.T
                    for c in range(NK // 512):
                        ps = kp_psp.tile([P, 512], F32, tag="kp")
                        for kt in range(KT):
                            nc.tensor.matmul(
                                ps,
                                wkT_sb[:, kt, h * P:(h + 1) * P],
                                kT_sb[:, kt, c * 512:(c + 1) * 512],
                                start=(kt == 0), stop=(kt == KT - 1),
                            )
                        nc.vector.tensor_copy(kpT_sb[:, h, c * 512:(c + 1) * 512], ps)

                    for c in range(RC):
                        rs = slice(c * 512, (c + 1) * 512)
                        att_ps = att_psp.tile([P, 512], F32, tag="att")
                        den_ps = den_psp.tile([P, 512], F32, tag="den")
                        exs = [None] * (2 * NSG)

                        def emit_lgexp(g):
                            lg_ps = lg_psp.tile([P, 2, 512], F32, tag="lg")
                            for j in range(2):
                                kkt = 2 * g + j
                                nc.tensor.matmul(
                                    lg_ps[:, j, :],
                                    kpT_sb[:, h, kkt * P:(kkt + 1) * P],
                                    xbf_sb[:, h, rs],
                                    start=True, stop=True,
                                )
                            ex = bsb.tile([P, 2, 512], BF16, tag="ex", bufs=4)
                            nc.scalar.activation(ex, lg_ps, Act.Exp)
                            exs[g] = ex

                        def emit_avden(sg):
                            for q in range(4):
                                kkt = 4 * sg + q
                                ex = exs[kkt // 2][:, kkt % 2, :]
                                nc.tensor.matmul(
                                    att_ps,
                                    vp_sb[:, kkt, h * DH:(h + 1) * DH],
                                    ex,
                                    start=(kkt == 0), stop=(kkt == KKT - 1),
                                )
                            for q in range(4):
                                kkt = 4 * sg + q
                                ex = exs[kkt // 2][:, kkt % 2, :]
                                nc.tensor.matmul(
                                    den_ps[32 * q:32 * (q + 1), :],
                                    maskones[:, kkt, :],
                                    ex,
                                    start=(sg == 0),
                                    stop=(sg == NSG - 1),
                                    tile_position=(0, 32 * q),
                                    skip_group_check=True,
                                )

                        emit_lgexp(0)
                        emit_lgexp(1)
                        emit_drain()  # previous iteration's normalize chain
                        for sg in range(1, NSG):
                            emit_lgexp(2 * sg)
                            emit_lgexp(2 * sg + 1)
                            emit_avden(sg - 1)
                        emit_avden(NSG - 1)
                        pending.append((h, rs, att_ps, den_ps))

                emit_drain()  # final iteration

        # ---------------- Phase C/D: LN1, MLP, LN2 ----------------
        with (
            tc.tile_pool(name="late", bufs=1) as late,
            tc.tile_pool(name="csb", bufs=1) as csb,
            tc.tile_pool(name="c_ps", bufs=2, space="PSUM") as c_ps,
        ):
            woT_sb = late.tile([P, KT, D], F32R)
            for t in range(KT):
                nc.sync.dma_start(out=woT_sb[:, t, :], in_=woT[t * P:(t + 1) * P, :])
            x1n_sb = late.tile([P, DT, RQ], F32)

            def ln_stats_rc(src, src_bf, c):
                """stats for one row-chunk: returns (mean_ps, rsg) both
                [P,512], identical across partitions."""
                rs = slice(c * 512, (c + 1) * 512)
                mean_ps = c_ps.tile([P, 512], F32, tag="mean")
                for kt in range(KT):
                    nc.tensor.matmul(
                        mean_ps, onesn, src_bf[:, kt, rs],
                        start=(kt == 0), stop=(kt == KT - 1),
                    )
                msq_ps = c_ps.tile([P, 512], F32, tag="msq")
                for kt in range(KT):
                    sq = csb.tile([P, 512], BF16, tag="sq", bufs=3)
                    nc.scalar.square(sq, src[:, kt, rs])
                    nc.tensor.matmul(
                        msq_ps, onesn, sq,
                        start=(kt == 0), stop=(kt == KT - 1),
                    )
                musq = csb.tile([P, 512], F32, tag="musq", bufs=2)
                nc.scalar.square(musq, mean_ps)
                var = csb.tile([P, 512], F32, tag="var", bufs=2)
                nc.vector.tensor_sub(var, msq_ps, musq)
                std = csb.tile([P, 512], F32, tag="std", bufs=2)
                nc.scalar.activation(std, var, Act.Sqrt, bias=eps_sb[:, :], scale=1.0)
                rsg = csb.tile([P, 512], F32, tag="rsg", bufs=2)
                nc.vector.reciprocal_approx_fast(rsg, std)
                return mean_ps, rsg

            # LN1: x1n = LN(x1) * g1 + b1   (ACT affine also rounds to f32r)
            for c in range(RC):
                rs = slice(c * 512, (c + 1) * 512)
                mean_ps, rsg = ln_stats_rc(qpT_sb, xbf_sb, c)
                for kt in range(DT):
                    xc = csb.tile([P, 512], F32, tag="xc", bufs=3)
                    nc.vector.tensor_sub(xc, qpT_sb[:, kt, rs], mean_ps)
                    xh = csb.tile([P, 512], F32, tag="xh", bufs=3)
                    nc.vector.tensor_mul(xh, xc, rsg)
                    nc.scalar.activation(
                        x1n_sb[:, kt, rs].bitcast(F32R), xh, Act.Identity,
                        bias=b1_sb[:, kt:kt + 1], scale=g1_sb[:, kt:kt + 1],
                    )

            # MLP: x2 = x1n + relu(Wout @ x1n.T + bout)  (x2 overwrites qpT),
            # then LN2 of that row-chunk with the affine folded:
            # out = x2*(rs*g2) - (mu*(rs*g2) - b2)
            for c in range(RC):
                rs = slice(c * 512, (c + 1) * 512)
                for dt_ in range(DT):
                    z_ps = c_ps.tile([P, 512], F32, tag="z")
                    for kt in range(KT):
                        nc.tensor.matmul(
                            z_ps,
                            woT_sb[:, kt, dt_ * P:(dt_ + 1) * P],
                            x1n_sb[:, kt, rs].bitcast(F32R),
                            start=(kt == 0), stop=(kt == KT - 1),
                        )
                    rl = csb.tile([P, 512], F32, tag="rl", bufs=3)
                    nc.scalar.activation(
                        rl, z_ps, Act.Relu,
                        bias=bo_sb[:, dt_:dt_ + 1], scale=1.0,
                    )
                    nc.vector.tensor_add(qpT_sb[:, dt_, rs], x1n_sb[:, dt_, rs], rl)
                    nc.vector.tensor_copy(xbf_sb[:, dt_, rs], qpT_sb[:, dt_, rs])

                mean_ps, rsg = ln_stats_rc(qpT_sb, xbf_sb, c)
                for kt in range(DT):
                    xc = csb.tile([P, 512], F32, tag="xc", bufs=3)
                    nc.vector.tensor_sub(xc, qpT_sb[:, kt, rs], mean_ps)
                    xh = csb.tile([P, 512], F32, tag="xh", bufs=3)
                    nc.vector.tensor_mul(xh, xc, rsg)
                    ot = csb.tile([P, 512], F32, tag="ot", bufs=3)
                    nc.scalar.activation(
                        ot, xh, Act.Identity,
                        bias=b2_sb[:, kt:kt + 1], scale=g2_sb[:, kt:kt + 1],
                    )
                    nc.sync.dma_start(out=outT[kt * P:(kt + 1) * P, rs], in_=ot)

    nc.compile()
    return nc


_NC_CACHE = None


def get_nc():
    global _NC_CACHE
    if _NC_CACHE is None:
        _NC_CACHE = build_nc()
    return _NC_CACHE


def shard_inputs(q, k, v, mask, Wq, Wk, Wv, Wout, bout, g1, b1, g2, b2):
    q = np.asarray(q, dtype=np.float32)
    k = np.asarray(k, dtype=np.float32)
    v = np.asarray(v, dtype=np.float32)
    mask = np.asarray(mask)
    f32c = lambda a: np.ascontiguousarray(np.asarray(a, dtype=np.float32))
    bfc = lambda a: np.ascontiguousarray(np.asarray(a, dtype=np.float32)).astype(BFNP)
    vec = lambda a: np.ascontiguousarray(
        np.asarray(a, dtype=np.float32).reshape(DT, P).T
    )

    shared = {
        "wqT": f32c(np.asarray(Wq, np.float32).T),
        "wkT": bfc(np.asarray(Wk, np.float32).T / np.sqrt(D)),
        "wvT": bfc(np.asarray(Wv, np.float32).T),
        "woT": f32c(np.asarray(Wout, np.float32).T),
        "g1": vec(g1), "b1": vec(b1), "g2": vec(g2), "b2": vec(b2),
        "bo": vec(bout),
    }
    in_maps = []
    for c in range(8):
        b, half = divmod(c, 2)
        rows = slice(half * RQ, (half + 1) * RQ)
        mb = (~mask[b]).astype(np.float32)  # 1.0 = keep, 0.0 = masked
        in_maps.append({
            "qT": f32c(q[b, rows].T),
            "kT": bfc(k[b].T),
            "vT": bfc(v[b].T),
            "maskb": np.ascontiguousarray(mb.reshape(KKT, P).T),
            **shared,
        })
    return in_maps


def assemble_output(results):
    out = np.empty((B, NQ, D), dtype=np.float32)
    for c in range(8):
        b, half = divmod(c, 2)
        rows = slice(half * RQ, (half + 1) * RQ)
        out[b, rows, :] = results[c]["outT"].T
    return out


def kernel(**inputs):
    nc = get_nc()
    in_maps = shard_inputs(**inputs)
    res = run_bass_kernel_spmd(nc, in_maps, core_ids=list(range(8)))
    return assemble_output(res.results)



# revision 3
# speedup vs baseline: 1.4863x; 1.4863x over previous
"""Trainium2 Bass kernel for nn_MultiHeadAttn (B=4, NQ=NK=2048, D=1024, H=8).

Sharding: 8 cores = 4 batches x 2 query-halves. Each core owns 1024 query rows
of one batch; k/v projections for that batch are computed redundantly by the
two cores sharing it (cheap after key compaction + fp8).

Key compaction: the mask is host-visible and ~50% of keys are masked
(their attention weight is exactly 0), so the host gathers the unmasked
keys per batch and pads to KCAP (multiple of 256). This halves kproj,
vproj, logits, A*V, den and the exp volume.

Precision: the attention branch is strongly attenuated in the output
(softmax over ~1K near-uniform keys -> att is ~3% of the residual qp), so
it runs in fp8e4m3 with DoubleRow matmuls (2 fp8 contraction elems/cell):
k, v, Wk*16, Wv*16, vp*16 and exp(logits) are fp8. The residual path
(qproj, MLP, layernorms) runs in bf16 with f32 accumulation.

Per-core dataflow (activations feature-major "T layout" [feat, row]):
  qpT = Wq @ qT            (bf16)
  vp  = v @ Wv.T * 16      (fp8 DoubleRow, natural [key, feat] layout, fp8 out)
  per head: kpT_h = (Wk*16) @ kT   (fp8 DoubleRow, bf16 out at 16x scale)
  per head, per 512-row chunk, over KCAP/256 key-tile pairs:
      logitsT[kk,r] = kpT_h_tile.T @ qpT_h    (bf16 matmul, PSUM f32)
      expT = Exp(logitsT / 512)               (ACT, fp8 out; /512 = /16/32)
      attT += vp_pair.T @ expT                (fp8 DoubleRow accumulate)
      den  += mones.T @ expT                  (fp8, 32-row col-tiled blocks)
  x1T = qpT + attT * (1/(16*den))             (the /16 undoes the vp scale)
  out1 = LN(x1) via ones-matmul stats (sums over feature partitions)
  x2T = out1 + Relu(Wout @ out1T + bout)      (bf16 matmul, ACT bias+relu)
  outT = LN(x2)  -> DRAM [feat, row]; host transposes back.
"""

from contextlib import ExitStack

import numpy as np
import ml_dtypes

import concourse.mybir as mybir
import concourse.tile as tile
from concourse import bacc
from concourse.bass_utils import run_bass_kernel_spmd

B, NQ, NK, D, H = 4, 2048, 2048, 1024, 8
DH = D // H            # 128, head dim
P = 128                # partitions
RQ = NQ // 2           # 1024 query rows per core
EPS = 1e-5

F32 = mybir.dt.float32
BF16 = mybir.dt.bfloat16
FP8 = mybir.dt.float8e4
BFNP = ml_dtypes.bfloat16
F8NP = ml_dtypes.float8_e4m3

KT = D // P            # 8 contraction tiles over features
DT = D // P            # 8 output-feature tiles (also heads)
RC = RQ // 512         # 2 row chunks of 512
DR = mybir.MatmulPerfMode.DoubleRow


def build_nc(kcap, debug=False):
    """kcap: padded (compacted) key count, multiple of 256."""
    assert kcap % 256 == 0
    KKT = kcap // 128          # key tiles
    KPAIR = KKT // 2           # DoubleRow key-tile pairs
    # key chunks for the kproj output (N dim), each <= 512
    kchunks = []
    o = 0
    while o < kcap:
        n = min(512, kcap - o)
        kchunks.append((o, n))
        o += n

    nc = bacc.Bacc("TRN2", target_bir_lowering=False, debug=debug)

    qT = nc.declare_dram_parameter("qT", [D, RQ], BF16, isOutput=False)
    kT = nc.declare_dram_parameter("kT", [D, kcap], FP8, isOutput=False)
    vT = nc.declare_dram_parameter("vT", [D, kcap], FP8, isOutput=False)
    wqT = nc.declare_dram_parameter("wqT", [D, D], BF16, isOutput=False)
    wkT = nc.declare_dram_parameter("wkT", [D, D], FP8, isOutput=False)
    wvT = nc.declare_dram_parameter("wvT", [D, D], FP8, isOutput=False)
    woT = nc.declare_dram_parameter("woT", [D, D], BF16, isOutput=False)
    mones = nc.declare_dram_parameter("mones", [P, KKT * 32], FP8, isOutput=False)
    g1 = nc.declare_dram_parameter("g1", [P, DT], F32, isOutput=False)
    b1 = nc.declare_dram_parameter("b1", [P, DT], F32, isOutput=False)
    g2 = nc.declare_dram_parameter("g2", [P, DT], F32, isOutput=False)
    b2 = nc.declare_dram_parameter("b2", [P, DT], F32, isOutput=False)
    bo = nc.declare_dram_parameter("bo", [P, DT], F32, isOutput=False)
    outT = nc.declare_dram_parameter("outT", [D, RQ], F32, isOutput=True)

    Act = mybir.ActivationFunctionType

    with tile.TileContext(nc) as tc, ExitStack() as ctx:
        consts = ctx.enter_context(tc.tile_pool(name="consts", bufs=1))
        pool_qp = ctx.enter_context(tc.tile_pool(name="pool_qp", bufs=1))

        onesn = consts.tile([P, P], BF16)
        nc.vector.memset(onesn, 1.0 / D)
        eps_sb = consts.tile([P, 1], F32)
        nc.vector.memset(eps_sb, EPS)
        # den-broadcast lhsT: the partition sum over dsb yields 32*den (4
        # blocks x 32 replicas); with weight 16/32 the result is 16*den, whose
        # reciprocal also undoes the vp*16 scale when multiplied into att.
        ones32 = consts.tile([P, P], BF16)
        nc.vector.memset(ones32, 16.0 / 32.0)
        mones_sb = consts.tile([P, KKT, 32], FP8)
        nc.sync.dma_start(out=mones_sb, in_=mones[:, :])
        g1_sb = consts.tile([P, DT], F32)
        nc.sync.dma_start(out=g1_sb, in_=g1[:, :])
        b1_sb = consts.tile([P, DT], F32)
        nc.sync.dma_start(out=b1_sb, in_=b1[:, :])
        g2_sb = consts.tile([P, DT], F32)
        nc.sync.dma_start(out=g2_sb, in_=g2[:, :])
        b2_sb = consts.tile([P, DT], F32)
        nc.sync.dma_start(out=b2_sb, in_=b2[:, :])
        bo_sb = consts.tile([P, DT], F32)
        nc.sync.dma_start(out=bo_sb, in_=bo[:, :])

        # persistent activations
        qpT_sb = pool_qp.tile([P, DT, RQ], F32)      # qp.T; becomes x1T then x2T
        xbf_sb = pool_qp.tile([P, DT, RQ], BF16)     # bf16 shadow (qp, then x1, x2)

        with (
            tc.tile_pool(name="pool_attn", bufs=1) as pool_attn,
            tc.tile_pool(name="pool_ain", bufs=1) as ain,
        ):
            kpT_sb = pool_attn.tile([P, H, kcap], BF16)  # per-head [dh, key], 16x
            vp_sb = pool_attn.tile([P, KKT, D], FP8)     # per key-tile [key, feat], 16x
            # ------------- Phase A: q and v projections ----------
            # k is projected per-head inside the attention section so PE work
            # interleaves with the ACT-bound exp stream.
            with tc.tile_pool(name="a_ps", bufs=3, space="PSUM") as a_ps:
                wqA_sb = ain.tile([P, KT, 512], BF16, tag="w")
                qT_sb = ain.tile([P, KT, RQ], BF16, tag="qk")
                for t in range(KT):
                    nc.sync.dma_start(out=wqA_sb[:, t, :], in_=wqT[t * P:(t + 1) * P, 0:512])
                    nc.sync.dma_start(out=qT_sb[:, t, 0:512], in_=qT[t * P:(t + 1) * P, 0:512])
                    nc.sync.dma_start(out=qT_sb[:, t, 512:1024], in_=qT[t * P:(t + 1) * P, 512:1024])
                wqB_sb = ain.tile([P, KT, 512], BF16, tag="w")
                for t in range(KT):
                    nc.sync.dma_start(out=wqB_sb[:, t, :], in_=wqT[t * P:(t + 1) * P, 512:1024])
                vT_sb = ain.tile([P, KT, kcap], FP8, tag="vv")
                for t in range(KT):
                    nc.sync.dma_start(out=vT_sb[:, t, :], in_=vT[t * P:(t + 1) * P, :])

                def q_proj(w_sb, dt0):
                    for dt_ in range(dt0, dt0 + 4):
                        for c in range(RC):
                            ps = a_ps.tile([P, 512], F32, tag="aps")
                            for kt in range(KT):
                                nc.tensor.matmul(
                                    ps,
                                    w_sb[:, kt, (dt_ - dt0) * P:(dt_ - dt0 + 1) * P],
                                    qT_sb[:, kt, c * 512:(c + 1) * 512],
                                    start=(kt == 0), stop=(kt == KT - 1),
                                )
                            nc.vector.tensor_copy(qpT_sb[:, dt_, c * 512:(c + 1) * 512], ps)
                            nc.vector.tensor_copy(xbf_sb[:, dt_, c * 512:(c + 1) * 512], ps)

                q_proj(wqA_sb, 0)
                q_proj(wqB_sb, 4)

                # v projection (fp8 DoubleRow): vp[kk, dout] = (v @ Wv.T) * 16
                # padded key rows are exactly zero (zero input columns).
                wvT_sb = ain.tile([P, KT, D], FP8, tag="w")
                for t in range(KT):
                    nc.sync.dma_start(out=wvT_sb[:, t, :], in_=wvT[t * P:(t + 1) * P, :])
                kT_sb = ain.tile([P, KT, kcap], FP8, tag="qk")
                for t in range(KT):
                    nc.sync.dma_start(out=kT_sb[:, t, :], in_=kT[t * P:(t + 1) * P, :])
                for kkt in range(KKT):
                    for c in range(D // 512):
                        ps = a_ps.tile([P, 512], F32, tag="aps")
                        for tp in range(KT // 2):
                            nc.tensor.matmul(
                                ps,
                                vT_sb[:, 2 * tp:2 * tp + 2, kkt * P:(kkt + 1) * P],
                                wvT_sb[:, 2 * tp:2 * tp + 2, c * 512:(c + 1) * 512],
                                start=(tp == 0), stop=(tp == KT // 2 - 1),
                                perf_mode=DR,
                            )
                        nc.vector.tensor_copy(vp_sb[:, kkt, c * 512:(c + 1) * 512], ps)

            wkT_sb = ain.tile([P, KT, D], FP8, tag="w")
            for t in range(KT):
                nc.sync.dma_start(out=wkT_sb[:, t, :], in_=wkT[t * P:(t + 1) * P, :])

            # den col-tiled block bookkeeping: block q accumulates kkt==q (mod 4)
            den_last = {q: max(k for k in range(KKT) if k % 4 == q) for q in range(4)}

            # ------------- Phase B: k projection + attention, per head -------
            # All retained keys are unmasked (masked keys were compacted away on
            # the host); zero-padded tail keys are excluded via zeroed vp rows
            # and zeroed den lhsT (mones). Emission is software-pipelined two
            # key-tile pairs ahead, and each (h,c) iteration's drain chain
            # (den sum -> reciprocal -> normalize -> residual) is deferred into
            # the next iteration so the in-order PE stream never waits.
            with (
                tc.tile_pool(name="att_ps", bufs=1, space="PSUM") as att_psp,
                tc.tile_pool(name="den_ps", bufs=1, space="PSUM") as den_psp,
                tc.tile_pool(name="lg_ps", bufs=2, space="PSUM") as lg_psp,
                tc.tile_pool(name="kp_ps", bufs=2, space="PSUM") as kp_psp,
                tc.tile_pool(name="bsb", bufs=1) as bsb,
            ):
                pending = []    # deferred drain state: (h, rs, att_ps, den_ps)

                def emit_drain():
                    if not pending:
                        return
                    h, rs, att_ps, den_ps = pending.pop()
                    # den blocks -> bf16 SBUF -> (1/512)-matmul sum + broadcast
                    dsb = bsb.tile([P, 512], BF16, tag="dsb", bufs=1)
                    nc.vector.tensor_copy(dsb, den_ps)
                    dbc = den_psp.tile([P, 512], F32, tag="den")
                    nc.tensor.matmul(dbc, ones32, dsb, start=True, stop=True)
                    rec = bsb.tile([P, 512], F32, tag="rec", bufs=1)
                    nc.vector.reciprocal_approx_fast(rec, dbc)
                    nc.vector.tensor_mul(rec, att_ps, rec)  # in-place att/(16 den)
                    # x1 = qp + att  (in place over qpT)
                    nc.vector.tensor_add(qpT_sb[:, h, rs], qpT_sb[:, h, rs], rec)
                    nc.vector.tensor_copy(xbf_sb[:, h, rs], qpT_sb[:, h, rs])

                for h in range(H):
                    # k projection for this head: kpT[h, :] = (Wk*16) @ k.T
                    for (co, cn) in kchunks:
                        ps = kp_psp.tile([P, 512], F32, tag="kp")
                        for tp in range(KT // 2):
                            nc.tensor.matmul(
                                ps[:, 0:cn],
                                wkT_sb[:, 2 * tp:2 * tp + 2, h * P:(h + 1) * P],
                                kT_sb[:, 2 * tp:2 * tp + 2, co:co + cn],
                                start=(tp == 0), stop=(tp == KT // 2 - 1),
                                perf_mode=DR,
                            )
                        nc.vector.tensor_copy(kpT_sb[:, h, co:co + cn], ps[:, 0:cn])

                    for c in range(RC):
                        rs = slice(c * 512, (c + 1) * 512)
                        att_ps = att_psp.tile([P, 512], F32, tag="att")
                        den_ps = den_psp.tile([P, 512], F32, tag="den")
                        exs = [None] * KPAIR

                        def emit_lgexp(g):
                            lg_ps = lg_psp.tile([P, 2, 512], F32, tag="lg")
                            for j in range(2):
                                kkt = 2 * g + j
                                nc.tensor.matmul(
                                    lg_ps[:, j, :],
                                    kpT_sb[:, h, kkt * P:(kkt + 1) * P],
                                    xbf_sb[:, h, rs],
                                    start=True, stop=True,
                                )
                            ex = bsb.tile([P, 2, 512], FP8, tag="ex", bufs=4)
                            # /512 = /16 (kp scale) /32 (sqrt(D))
                            nc.scalar.activation(ex, lg_ps, Act.Exp, scale=1.0 / 512.0)
                            exs[g] = ex

                        def emit_avden(g):
                            nc.tensor.matmul(
                                att_ps,
                                vp_sb[:, 2 * g:2 * g + 2, h * DH:(h + 1) * DH],
                                exs[g],
                                start=(g == 0), stop=(g == KPAIR - 1),
                                perf_mode=DR,
                            )
                            for j in range(2):
                                kkt = 2 * g + j
                                q = kkt % 4
                                nc.tensor.matmul(
                                    den_ps[32 * q:32 * (q + 1), :],
                                    mones_sb[:, kkt, :],
                                    exs[g][:, j, :],
                                    start=(kkt < 4),
                                    stop=(kkt == den_last[q]),
                                    tile_position=(0, 32 * q),
                                    skip_group_check=True,
                                )

                        emit_lgexp(0)
                        emit_lgexp(1)
                        emit_drain()  # previous iteration's normalize chain
                        for g in range(2, KPAIR):
                            emit_lgexp(g)
                            emit_avden(g - 2)
                        emit_avden(KPAIR - 2)
                        emit_avden(KPAIR - 1)
                        pending.append((h, rs, att_ps, den_ps))

                emit_drain()  # final iteration

        # ---------------- Phase C/D: LN1, MLP, LN2 ----------------
        with (
            tc.tile_pool(name="late", bufs=1) as late,
            tc.tile_pool(name="csb", bufs=1) as csb,
            tc.tile_pool(name="c_ps", bufs=2, space="PSUM") as c_ps,
        ):
            woT_sb = late.tile([P, KT, D], BF16)
            for t in range(KT):
                nc.sync.dma_start(out=woT_sb[:, t, :], in_=woT[t * P:(t + 1) * P, :])
            x1n_sb = late.tile([P, DT, RQ], BF16)

            def ln_stats_rc(src, src_bf, c):
                """stats for one row-chunk: returns (mean_ps, rsg) both
                [P,512], identical across partitions."""
                rs = slice(c * 512, (c + 1) * 512)
                mean_ps = c_ps.tile([P, 512], F32, tag="mean")
                for kt in range(KT):
                    nc.tensor.matmul(
                        mean_ps, onesn, src_bf[:, kt, rs],
                        start=(kt == 0), stop=(kt == KT - 1),
                    )
                msq_ps = c_ps.tile([P, 512], F32, tag="msq")
                for kt in range(KT):
                    sq = csb.tile([P, 512], BF16, tag="sq", bufs=3)
                    nc.scalar.square(sq, src[:, kt, rs])
                    nc.tensor.matmul(
                        msq_ps, onesn, sq,
                        start=(kt == 0), stop=(kt == KT - 1),
                    )
                musq = csb.tile([P, 512], F32, tag="musq", bufs=2)
                nc.scalar.square(musq, mean_ps)
                var = csb.tile([P, 512], F32, tag="var", bufs=2)
                nc.vector.tensor_sub(var, msq_ps, musq)
                std = csb.tile([P, 512], F32, tag="std", bufs=2)
                nc.scalar.activation(std, var, Act.Sqrt, bias=eps_sb[:, :], scale=1.0)
                rsg = csb.tile([P, 512], F32, tag="rsg", bufs=2)
                nc.vector.reciprocal_approx_fast(rsg, std)
                return mean_ps, rsg

            # LN1: x1n = LN(x1) * g1 + b1   (ACT affine, bf16 out)
            for c in range(RC):
                rs = slice(c * 512, (c + 1) * 512)
                mean_ps, rsg = ln_stats_rc(qpT_sb, xbf_sb, c)
                for kt in range(DT):
                    xc = csb.tile([P, 512], F32, tag="xc", bufs=3)
                    nc.vector.tensor_sub(xc, qpT_sb[:, kt, rs], mean_ps)
                    xh = csb.tile([P, 512], F32, tag="xh", bufs=3)
                    nc.vector.tensor_mul(xh, xc, rsg)
                    nc.scalar.activation(
                        x1n_sb[:, kt, rs], xh, Act.Identity,
                        bias=b1_sb[:, kt:kt + 1], scale=g1_sb[:, kt:kt + 1],
                    )

            # MLP: x2 = x1n + relu(Wout @ x1n.T + bout)  (x2 overwrites qpT),
            # then LN2 of that row-chunk
            for c in range(RC):
                rs = slice(c * 512, (c + 1) * 512)
                for dt_ in range(DT):
                    z_ps = c_ps.tile([P, 512], F32, tag="z")
                    for kt in range(KT):
                        nc.tensor.matmul(
                            z_ps,
                            woT_sb[:, kt, dt_ * P:(dt_ + 1) * P],
                            x1n_sb[:, kt, rs],
                            start=(kt == 0), stop=(kt == KT - 1),
                        )
                    rl = csb.tile([P, 512], F32, tag="rl", bufs=3)
                    nc.scalar.activation(
                        rl, z_ps, Act.Relu,
                        bias=bo_sb[:, dt_:dt_ + 1], scale=1.0,
                    )
                    nc.vector.tensor_add(qpT_sb[:, dt_, rs], x1n_sb[:, dt_, rs], rl)
                    nc.vector.tensor_copy(xbf_sb[:, dt_, rs], qpT_sb[:, dt_, rs])

                mean_ps, rsg = ln_stats_rc(qpT_sb, xbf_sb, c)
                for kt in range(DT):
                    xc = csb.tile([P, 512], F32, tag="xc", bufs=3)
                    nc.vector.tensor_sub(xc, qpT_sb[:, kt, rs], mean_ps)
                    xh = csb.tile([P, 512], F32, tag="xh", bufs=3)
                    nc.vector.tensor_mul(xh, xc, rsg)
                    ot = csb.tile([P, 512], F32, tag="ot", bufs=3)
                    nc.scalar.activation(
                        ot, xh, Act.Identity,
                        bias=b2_sb[:, kt:kt + 1], scale=g2_sb[:, kt:kt + 1],
                    )
                    nc.sync.dma_start(out=outT[kt * P:(kt + 1) * P, rs], in_=ot)

    nc.compile()
    return nc


_NC_CACHE = {}


def get_nc(kcap=1280, debug=False):
    key = (kcap, debug)
    if key not in _NC_CACHE:
        _NC_CACHE[key] = build_nc(kcap, debug=debug)
    return _NC_CACHE[key]


def choose_kcap(mask):
    nkeep = int((~np.asarray(mask)).sum(axis=1).max())
    return max(256, -(-nkeep // 256) * 256)


def shard_inputs(q, k, v, mask, Wq, Wk, Wv, Wout, bout, g1, b1, g2, b2,
                 kcap=None):
    q = np.asarray(q, dtype=np.float32)
    k = np.asarray(k, dtype=np.float32)
    v = np.asarray(v, dtype=np.float32)
    mask = np.asarray(mask)
    if kcap is None:
        kcap = choose_kcap(mask)
    KKT = kcap // 128
    bfc = lambda a: np.ascontiguousarray(np.asarray(a, dtype=np.float32)).astype(BFNP)
    f8c = lambda a: np.ascontiguousarray(np.asarray(a, dtype=np.float32)).astype(F8NP)
    vec = lambda a: np.ascontiguousarray(
        np.asarray(a, dtype=np.float32).reshape(DT, P).T
    )

    shared = {
        "wqT": bfc(np.asarray(Wq, np.float32).T),
        "wkT": f8c(np.asarray(Wk, np.float32).T * 16.0),
        "wvT": f8c(np.asarray(Wv, np.float32).T * 16.0),
        "woT": bfc(np.asarray(Wout, np.float32).T),
        "g1": vec(g1), "b1": vec(b1), "g2": vec(g2), "b2": vec(b2),
        "bo": vec(bout),
    }
    in_maps = []
    for bi in range(B):
        keep = np.where(~mask[bi])[0]
        nk = len(keep)
        kc = np.zeros((D, kcap), np.float32)
        vc = np.zeros((D, kcap), np.float32)
        kc[:, :nk] = k[bi][keep].T
        vc[:, :nk] = v[bi][keep].T
        mo = np.zeros((kcap, 32), np.float32)   # [key, 32] -> [P, KKT*32]
        mo[:nk] = 1.0
        mo = mo.reshape(KKT, P, 32).transpose(1, 0, 2).reshape(P, KKT * 32)
        per_batch = {
            "kT": f8c(kc),
            "vT": f8c(vc),
            "mones": f8c(mo),
            **shared,
        }
        for half in range(2):
            rows = slice(half * RQ, (half + 1) * RQ)
            in_maps.append({
                "qT": bfc(q[bi, rows].T),
                **per_batch,
            })
    return in_maps


def assemble_output(results):
    out = np.empty((B, NQ, D), dtype=np.float32)
    for c in range(8):
        bi, half = divmod(c, 2)
        rows = slice(half * RQ, (half + 1) * RQ)
        out[bi, rows, :] = results[c]["outT"].T
    return out


def kernel(**inputs):
    kcap = choose_kcap(inputs["mask"])
    nc = get_nc(kcap)
    in_maps = shard_inputs(**inputs, kcap=kcap)
    res = run_bass_kernel_spmd(nc, in_maps, core_ids=list(range(8)))
    return assemble_output(res.results)


# revision 12
# speedup vs baseline: 1.5656x; 1.0534x over previous
"""Trainium2 Bass kernel for nn_MultiHeadAttn (B=4, NQ=NK=2048, D=1024, H=8).

Sharding: 8 cores = 4 batches x 2 query-halves. Each core owns 1024 query rows
of one batch; k/v projections for that batch are computed redundantly by the
two cores sharing it (cheap after key compaction + fp8).

Key compaction: the mask is host-visible and ~50% of keys are masked
(their attention weight is exactly 0), so the host gathers the unmasked
keys per batch and pads to KCAP (multiple of 256). This halves kproj,
vproj, logits, A*V, den and the exp volume.

Precision: the attention branch is strongly attenuated in the output
(softmax over ~1K near-uniform keys -> att is ~3% of the residual qp), so
it runs in fp8e4m3 with DoubleRow matmuls (2 fp8 contraction elems/cell):
k, v, Wk*16, Wv*16, vp*16 and exp(logits) are fp8. The residual path
(qproj, MLP, layernorms) runs in bf16 with f32 accumulation.

Per-core dataflow (activations feature-major "T layout" [feat, row]):
  qpT = Wq @ qT            (bf16)
  vp  = v @ Wv.T * 16      (fp8 DoubleRow, natural [key, feat] layout, fp8 out)
  per head: kpT_h = (Wk*16) @ kT   (fp8 DoubleRow, bf16 out at 16x scale)
  per head, per 512-row chunk, over KCAP/256 key-tile pairs:
      logitsT[kk,r] = kpT_h_tile.T @ qpT_h    (bf16 matmul, PSUM f32)
      expT = Exp(logitsT / 512)               (ACT, fp8 out; /512 = /16/32)
      attT += vp_pair.T @ expT                (fp8 DoubleRow accumulate)
      den  += mones.T @ expT                  (fp8, 32-row col-tiled blocks)
  x1T = qpT + attT * (1/(16*den))             (the /16 undoes the vp scale)
  out1 = LN(x1) via ones-matmul stats (sums over feature partitions)
  x2T = out1 + Relu(Wout @ out1T + bout)      (bf16 matmul, ACT bias+relu)
  outT = LN(x2)  -> DRAM [feat, row]; host transposes back.
"""

from contextlib import ExitStack

import numpy as np
import ml_dtypes

import concourse.mybir as mybir
import concourse.tile as tile
from concourse import bacc
from concourse.bass_utils import run_bass_kernel_spmd

B, NQ, NK, D, H = 4, 2048, 2048, 1024, 8
DH = D // H            # 128, head dim
P = 128                # partitions
RQ = NQ // 2           # 1024 query rows per core
EPS = 1e-5

F32 = mybir.dt.float32
BF16 = mybir.dt.bfloat16
FP8 = mybir.dt.float8e4
BFNP = ml_dtypes.bfloat16
F8NP = ml_dtypes.float8_e4m3

KT = D // P            # 8 contraction tiles over features
DT = D // P            # 8 output-feature tiles (also heads)
RC = RQ // 512         # 2 row chunks of 512
DR = mybir.MatmulPerfMode.DoubleRow


def build_nc(kcap, debug=False):
    """kcap: padded (compacted) key count, multiple of 256."""
    assert kcap % 256 == 0
    KKT = kcap // 128          # key tiles
    KPAIR = KKT // 2           # DoubleRow key-tile pairs
    # key chunks for the kproj output (N dim), each <= 512
    kchunks = []
    o = 0
    while o < kcap:
        n = min(512, kcap - o)
        kchunks.append((o, n))
        o += n

    nc = bacc.Bacc("TRN2", target_bir_lowering=False, debug=debug)

    qT = nc.declare_dram_parameter("qT", [D, RQ], BF16, isOutput=False)
    kT = nc.declare_dram_parameter("kT", [D, kcap], FP8, isOutput=False)
    vT = nc.declare_dram_parameter("vT", [D, kcap], FP8, isOutput=False)
    wqT = nc.declare_dram_parameter("wqT", [D, D], BF16, isOutput=False)
    wkT = nc.declare_dram_parameter("wkT", [D, D], FP8, isOutput=False)
    wvT = nc.declare_dram_parameter("wvT", [D, D], FP8, isOutput=False)
    woT = nc.declare_dram_parameter("woT", [D, D], BF16, isOutput=False)
    mones = nc.declare_dram_parameter("mones", [P, KKT * 32], FP8, isOutput=False)
    g1 = nc.declare_dram_parameter("g1", [P, DT], F32, isOutput=False)
    b1 = nc.declare_dram_parameter("b1", [P, DT], F32, isOutput=False)
    g2 = nc.declare_dram_parameter("g2", [P, DT], F32, isOutput=False)
    b2 = nc.declare_dram_parameter("b2", [P, DT], F32, isOutput=False)
    bo = nc.declare_dram_parameter("bo", [P, DT], F32, isOutput=False)
    outT = nc.declare_dram_parameter("outT", [D, RQ], F32, isOutput=True)

    Act = mybir.ActivationFunctionType

    with tile.TileContext(nc) as tc, ExitStack() as ctx:
        consts = ctx.enter_context(tc.tile_pool(name="consts", bufs=1))
        pool_qp = ctx.enter_context(tc.tile_pool(name="pool_qp", bufs=1))

        onesn = consts.tile([P, P], BF16)
        nc.vector.memset(onesn, 1.0 / D)
        eps_sb = consts.tile([P, 1], F32)
        nc.vector.memset(eps_sb, EPS)
        # den-broadcast lhsT: the partition sum over dsb yields 32*den (4
        # blocks x 32 replicas); with weight 16/32 the result is 16*den, whose
        # reciprocal also undoes the vp*16 scale when multiplied into att.
        ones32 = consts.tile([P, P], BF16)
        nc.vector.memset(ones32, 16.0 / 32.0)
        mones_sb = consts.tile([P, KKT, 32], FP8)
        nc.sync.dma_start(out=mones_sb, in_=mones[:, :])
        g1_sb = consts.tile([P, DT], F32)
        nc.sync.dma_start(out=g1_sb, in_=g1[:, :])
        b1_sb = consts.tile([P, DT], F32)
        nc.sync.dma_start(out=b1_sb, in_=b1[:, :])
        g2_sb = consts.tile([P, DT], F32)
        nc.sync.dma_start(out=g2_sb, in_=g2[:, :])
        b2_sb = consts.tile([P, DT], F32)
        nc.sync.dma_start(out=b2_sb, in_=b2[:, :])
        bo_sb = consts.tile([P, DT], F32)
        nc.sync.dma_start(out=bo_sb, in_=bo[:, :])

        # persistent activations
        qpT_sb = pool_qp.tile([P, DT, RQ], F32)      # qp.T; becomes x1T then x2T
        xbf_sb = pool_qp.tile([P, DT, RQ], BF16)     # bf16 shadow (qp, then x1, x2)

        with (
            tc.tile_pool(name="pool_attn", bufs=1) as pool_attn,
            tc.tile_pool(name="pool_ain", bufs=1) as ain,
        ):
            kpT_sb = pool_attn.tile([P, H, kcap], BF16)  # per-head [dh, key], 16x
            vp_sb = pool_attn.tile([P, KKT, D], FP8)     # per key-tile [key, feat], 16x
            # ------------- Phase A: q and v projections ----------
            # k is projected per-head inside the attention section so PE work
            # interleaves with the ACT-bound exp stream.
            with tc.tile_pool(name="a_ps", bufs=3, space="PSUM") as a_ps:
                # per-kt tiles so the first matmul only waits on its own slice
                wqA_t = [ain.tile([P, 512], BF16, tag=f"wqa{t}", name=f"wqA{t}")
                         for t in range(KT)]
                qT_t = [ain.tile([P, RQ], BF16, tag=f"qt{t}", name=f"qTs{t}")
                        for t in range(KT)]
                for t in range(KT):
                    nc.sync.dma_start(out=wqA_t[t], in_=wqT[t * P:(t + 1) * P, 0:512])
                    nc.sync.dma_start(out=qT_t[t][:, 0:512], in_=qT[t * P:(t + 1) * P, 0:512])
                    nc.sync.dma_start(out=qT_t[t][:, 512:1024], in_=qT[t * P:(t + 1) * P, 512:1024])
                wqB_t = [ain.tile([P, 512], BF16, tag=f"wqa{t}", name=f"wqB{t}")
                         for t in range(KT)]
                for t in range(KT):
                    nc.sync.dma_start(out=wqB_t[t], in_=wqT[t * P:(t + 1) * P, 512:1024])
                vT_sb = ain.tile([P, KT, kcap], FP8, tag="vv")
                for t in range(KT):
                    nc.sync.dma_start(out=vT_sb[:, t, :], in_=vT[t * P:(t + 1) * P, :])

                def q_proj(w_t, dt0):
                    for dt_ in range(dt0, dt0 + 4):
                        for c in range(RC):
                            ps = a_ps.tile([P, 512], F32, tag="aps")
                            for kt in range(KT):
                                nc.tensor.matmul(
                                    ps,
                                    w_t[kt][:, (dt_ - dt0) * P:(dt_ - dt0 + 1) * P],
                                    qT_t[kt][:, c * 512:(c + 1) * 512],
                                    start=(kt == 0), stop=(kt == KT - 1),
                                )
                            nc.vector.tensor_copy(qpT_sb[:, dt_, c * 512:(c + 1) * 512], ps)
                            nc.vector.tensor_copy(xbf_sb[:, dt_, c * 512:(c + 1) * 512], ps)

                q_proj(wqA_t, 0)
                q_proj(wqB_t, 4)

                # v projection (fp8 DoubleRow): vp[kk, dout] = (v @ Wv.T) * 16
                # padded key rows are exactly zero (zero input columns).
                wvT_sb = ain.tile([P, KT, D], FP8, tag="w")
                for t in range(KT):
                    nc.sync.dma_start(out=wvT_sb[:, t, :], in_=wvT[t * P:(t + 1) * P, :])
                kT_sb = ain.tile([P, KT, kcap], FP8, tag="qk")
                for t in range(KT):
                    nc.sync.dma_start(out=kT_sb[:, t, :], in_=kT[t * P:(t + 1) * P, :])
                for kkt in range(KKT):
                    for c in range(D // 512):
                        ps = a_ps.tile([P, 512], F32, tag="aps")
                        for tp in range(KT // 2):
                            nc.tensor.matmul(
                                ps,
                                vT_sb[:, 2 * tp:2 * tp + 2, kkt * P:(kkt + 1) * P],
                                wvT_sb[:, 2 * tp:2 * tp + 2, c * 512:(c + 1) * 512],
                                start=(tp == 0), stop=(tp == KT // 2 - 1),
                                perf_mode=DR,
                            )
                        nc.vector.tensor_copy(vp_sb[:, kkt, c * 512:(c + 1) * 512], ps)

            wkT_sb = ain.tile([P, KT, D], FP8, tag="wk")
            for t in range(KT):
                nc.sync.dma_start(out=wkT_sb[:, t, :], in_=wkT[t * P:(t + 1) * P, :])

            # den col-tiled block bookkeeping: block q accumulates kkt==q (mod 4)
            den_last = {q: max(k for k in range(KKT) if k % 4 == q) for q in range(4)}

            # ------------- Phase B: k projection + attention, per head -------
            # All retained keys are unmasked (masked keys were compacted away on
            # the host); zero-padded tail keys are excluded via zeroed vp rows
            # and zeroed den lhsT (mones). Emission is software-pipelined two
            # key-tile pairs ahead, and each (h,c) iteration's drain chain
            # (den sum -> reciprocal -> normalize -> residual) is deferred into
            # the next iteration so the in-order PE stream never waits.
            with (
                tc.tile_pool(name="att_ps", bufs=1, space="PSUM") as att_psp,
                tc.tile_pool(name="den_ps", bufs=1, space="PSUM") as den_psp,
                tc.tile_pool(name="lg_ps", bufs=2, space="PSUM") as lg_psp,
                tc.tile_pool(name="kp_ps", bufs=2, space="PSUM") as kp_psp,
                tc.tile_pool(name="bsb", bufs=1) as bsb,
            ):
                pending = []    # deferred drain state: (h, rs, att_ps, den_ps)

                def emit_drain():
                    if not pending:
                        return
                    h, rs, att_ps, den_ps = pending.pop()
                    # den blocks -> bf16 SBUF -> (1/512)-matmul sum + broadcast
                    dsb = bsb.tile([P, 512], BF16, tag="dsb", bufs=1)
                    nc.vector.tensor_copy(dsb, den_ps)
                    dbc = den_psp.tile([P, 512], F32, tag="den")
                    nc.tensor.matmul(dbc, ones32, dsb, start=True, stop=True)
                    rec = bsb.tile([P, 512], F32, tag="rec", bufs=1)
                    nc.vector.reciprocal_approx_fast(rec, dbc)
                    nc.vector.tensor_mul(rec, att_ps, rec)  # in-place att/(16 den)
                    # x1 = qp + att  (in place over qpT)
                    nc.vector.tensor_add(qpT_sb[:, h, rs], qpT_sb[:, h, rs], rec)
                    nc.vector.tensor_copy(xbf_sb[:, h, rs], qpT_sb[:, h, rs])

                for h in range(H):
                    # k projection for this head: kpT[h, :] = (Wk*16) @ k.T
                    for (co, cn) in kchunks:
                        ps = kp_psp.tile([P, 512], F32, tag="kp")
                        for tp in range(KT // 2):
                            nc.tensor.matmul(
                                ps[:, 0:cn],
                                wkT_sb[:, 2 * tp:2 * tp + 2, h * P:(h + 1) * P],
                                kT_sb[:, 2 * tp:2 * tp + 2, co:co + cn],
                                start=(tp == 0), stop=(tp == KT // 2 - 1),
                                perf_mode=DR,
                            )
                        nc.vector.tensor_copy(kpT_sb[:, h, co:co + cn], ps[:, 0:cn])

                    for c in range(RC):
                        rs = slice(c * 512, (c + 1) * 512)
                        att_ps = att_psp.tile([P, 512], F32, tag="att")
                        den_ps = den_psp.tile([P, 512], F32, tag="den")
                        exs = [None] * KPAIR

                        def emit_lgexp(g):
                            lg_ps = lg_psp.tile([P, 2, 512], F32, tag="lg")
                            for j in range(2):
                                kkt = 2 * g + j
                                nc.tensor.matmul(
                                    lg_ps[:, j, :],
                                    kpT_sb[:, h, kkt * P:(kkt + 1) * P],
                                    xbf_sb[:, h, rs],
                                    start=True, stop=True,
                                )
                            ex = bsb.tile([P, 2, 512], FP8, tag="ex", bufs=KPAIR + 1)
                            # /512 = /16 (kp scale) /32 (sqrt(D))
                            nc.scalar.activation(ex, lg_ps, Act.Exp, scale=1.0 / 512.0)
                            exs[g] = ex

                        def emit_att(g):
                            nc.tensor.matmul(
                                att_ps,
                                vp_sb[:, 2 * g:2 * g + 2, h * DH:(h + 1) * DH],
                                exs[g],
                                start=(g == 0), stop=(g == KPAIR - 1),
                                perf_mode=DR,
                            )

                        def emit_den(k0, k1):
                            # den matmuls back-to-back so the 32-col tiles
                            # pack concurrently in the array
                            for kkt in range(k0, k1):
                                q = kkt % 4
                                nc.tensor.matmul(
                                    den_ps[32 * q:32 * (q + 1), :],
                                    mones_sb[:, kkt, :],
                                    exs[kkt // 2][:, kkt % 2, :],
                                    start=(kkt < 4),
                                    stop=(kkt == den_last[q]),
                                    tile_position=(0, 32 * q),
                                    skip_group_check=True,
                                )

                        emit_lgexp(0)
                        emit_lgexp(1)
                        emit_drain()  # previous iteration's normalize chain
                        den_done = 0
                        for g in range(2, KPAIR):
                            emit_lgexp(g)
                            emit_att(g - 2)
                            # den for 4 key tiles at a time, two ex-pairs back
                            if 2 * (g - 1) - den_done >= 4:
                                emit_den(den_done, den_done + 4)
                                den_done += 4
                        emit_att(KPAIR - 2)
                        emit_att(KPAIR - 1)
                        emit_den(den_done, KKT)
                        pending.append((h, rs, att_ps, den_ps))

                emit_drain()  # final iteration

        # ---------------- Phase C/D: LN1, MLP, LN2 ----------------
        with (
            tc.tile_pool(name="late", bufs=1) as late,
            tc.tile_pool(name="csb", bufs=1) as csb,
            tc.tile_pool(name="c_ps", bufs=2, space="PSUM") as c_ps,
        ):
            woT_sb = late.tile([P, KT, D], BF16)
            for t in range(KT):
                nc.sync.dma_start(out=woT_sb[:, t, :], in_=woT[t * P:(t + 1) * P, :])
            x1n_sb = late.tile([P, DT, RQ], BF16)

            def ln_stats_rc(src, mean_srcs, c):
                """stats for one row-chunk: returns (mean_ps, rsg) both
                [P,512], identical across partitions. mean_srcs: list of bf16
                [P,512] APs whose per-feature sum is the row vector."""
                rs = slice(c * 512, (c + 1) * 512)
                mean_ps = c_ps.tile([P, 512], F32, tag="mean")
                for i, ms in enumerate(mean_srcs):
                    nc.tensor.matmul(
                        mean_ps, onesn, ms,
                        start=(i == 0), stop=(i == len(mean_srcs) - 1),
                    )
                msq_ps = c_ps.tile([P, 512], F32, tag="msq")
                for kt in range(KT):
                    sq = csb.tile([P, 512], BF16, tag="sq", bufs=3)
                    nc.scalar.square(sq, src[:, kt, rs])
                    nc.tensor.matmul(
                        msq_ps, onesn, sq,
                        start=(kt == 0), stop=(kt == KT - 1),
                    )
                musq = csb.tile([P, 512], F32, tag="musq", bufs=2)
                nc.scalar.square(musq, mean_ps)
                var = csb.tile([P, 512], F32, tag="var", bufs=2)
                nc.vector.tensor_sub(var, msq_ps, musq)
                std = csb.tile([P, 512], F32, tag="std", bufs=2)
                nc.scalar.activation(std, var, Act.Sqrt, bias=eps_sb[:, :], scale=1.0)
                rsg = csb.tile([P, 512], F32, tag="rsg", bufs=2)
                nc.vector.reciprocal_approx_fast(rsg, std)
                return mean_ps, rsg

            def normalize(src, mean_ps, rsg, emit_out, rs):
                # (x - mean) * rsg per feature tile; sub/mul alternate between
                # DVE and GpSimd to split the element-wise load.
                for kt in range(DT):
                    eng = nc.vector
                    xc = csb.tile([P, 512], F32, tag="xc", bufs=4)
                    eng.tensor_sub(xc, src[:, kt, rs], mean_ps)
                    xh = csb.tile([P, 512], F32, tag="xh", bufs=4)
                    eng.tensor_mul(xh, xc, rsg)
                    emit_out(kt, xh)

            # LN1: x1n = LN(x1) * g1 + b1   (ACT affine, bf16 out)
            for c in range(RC):
                rs = slice(c * 512, (c + 1) * 512)
                mean_ps, rsg = ln_stats_rc(
                    qpT_sb, [xbf_sb[:, kt, rs] for kt in range(KT)], c)

                def ln1_out(kt, xh, rs=rs):
                    nc.scalar.activation(
                        x1n_sb[:, kt, rs], xh, Act.Identity,
                        bias=b1_sb[:, kt:kt + 1], scale=g1_sb[:, kt:kt + 1],
                    )
                normalize(qpT_sb, mean_ps, rsg, ln1_out, rs)

            # MLP: x2 = x1n + relu(Wout @ x1n.T + bout)  (x2 overwrites qpT),
            # then LN2 of that row-chunk. The LN2 mean is accumulated from the
            # x1n and relu bf16 tiles directly (no x2 bf16 shadow needed).
            for c in range(RC):
                rs = slice(c * 512, (c + 1) * 512)
                rls = []
                for dt_ in range(DT):
                    z_ps = c_ps.tile([P, 512], F32, tag="z")
                    for kt in range(KT):
                        nc.tensor.matmul(
                            z_ps,
                            woT_sb[:, kt, dt_ * P:(dt_ + 1) * P],
                            x1n_sb[:, kt, rs],
                            start=(kt == 0), stop=(kt == KT - 1),
                        )
                    rl = csb.tile([P, 512], BF16, tag="rl", bufs=DT)
                    nc.scalar.activation(
                        rl, z_ps, Act.Relu,
                        bias=bo_sb[:, dt_:dt_ + 1], scale=1.0,
                    )
                    nc.vector.tensor_add(qpT_sb[:, dt_, rs], x1n_sb[:, dt_, rs], rl)
                    rls.append(rl)

                mean_ps, rsg = ln_stats_rc(
                    qpT_sb, [x1n_sb[:, kt, rs] for kt in range(KT)] + rls, c)

                def ln2_out(kt, xh, rs=rs):
                    ot = csb.tile([P, 512], F32, tag="ot", bufs=3)
                    nc.scalar.activation(
                        ot, xh, Act.Identity,
                        bias=b2_sb[:, kt:kt + 1], scale=g2_sb[:, kt:kt + 1],
                    )
                    nc.sync.dma_start(out=outT[kt * P:(kt + 1) * P, rs], in_=ot)
                normalize(qpT_sb, mean_ps, rsg, ln2_out, rs)

    nc.compile()
    return nc


_NC_CACHE = {}


def get_nc(kcap=1280, debug=False):
    key = (kcap, debug)
    if key not in _NC_CACHE:
        _NC_CACHE[key] = build_nc(kcap, debug=debug)
    return _NC_CACHE[key]


def choose_kcap(mask):
    nkeep = int((~np.asarray(mask)).sum(axis=1).max())
    return max(256, -(-nkeep // 256) * 256)


def shard_inputs(q, k, v, mask, Wq, Wk, Wv, Wout, bout, g1, b1, g2, b2,
                 kcap=None):
    q = np.asarray(q, dtype=np.float32)
    k = np.asarray(k, dtype=np.float32)
    v = np.asarray(v, dtype=np.float32)
    mask = np.asarray(mask)
    if kcap is None:
        kcap = choose_kcap(mask)
    KKT = kcap // 128
    bfc = lambda a: np.ascontiguousarray(np.asarray(a, dtype=np.float32)).astype(BFNP)
    f8c = lambda a: np.ascontiguousarray(np.asarray(a, dtype=np.float32)).astype(F8NP)
    vec = lambda a: np.ascontiguousarray(
        np.asarray(a, dtype=np.float32).reshape(DT, P).T
    )

    shared = {
        "wqT": bfc(np.asarray(Wq, np.float32).T),
        "wkT": f8c(np.asarray(Wk, np.float32).T * 16.0),
        "wvT": f8c(np.asarray(Wv, np.float32).T * 16.0),
        "woT": bfc(np.asarray(Wout, np.float32).T),
        "g1": vec(g1), "b1": vec(b1), "g2": vec(g2), "b2": vec(b2),
        "bo": vec(bout),
    }
    in_maps = []
    for bi in range(B):
        keep = np.where(~mask[bi])[0]
        nk = len(keep)
        kc = np.zeros((D, kcap), np.float32)
        vc = np.zeros((D, kcap), np.float32)
        kc[:, :nk] = k[bi][keep].T
        vc[:, :nk] = v[bi][keep].T
        mo = np.zeros((kcap, 32), np.float32)   # [key, 32] -> [P, KKT*32]
        mo[:nk] = 1.0
        mo = mo.reshape(KKT, P, 32).transpose(1, 0, 2).reshape(P, KKT * 32)
        per_batch = {
            "kT": f8c(kc),
            "vT": f8c(vc),
            "mones": f8c(mo),
            **shared,
        }
        for half in range(2):
            rows = slice(half * RQ, (half + 1) * RQ)
            in_maps.append({
                "qT": bfc(q[bi, rows].T),
                **per_batch,
            })
    return in_maps


def assemble_output(results):
    out = np.empty((B, NQ, D), dtype=np.float32)
    for c in range(8):
        bi, half = divmod(c, 2)
        rows = slice(half * RQ, (half + 1) * RQ)
        out[bi, rows, :] = results[c]["outT"].T
    return out


def kernel(**inputs):
    kcap = choose_kcap(inputs["mask"])
    nc = get_nc(kcap)
    in_maps = shard_inputs(**inputs, kcap=kcap)
    res = run_bass_kernel_spmd(nc, in_maps, core_ids=list(range(8)))
    return assemble_output(res.results)


# revision 19
# speedup vs baseline: 1.5782x; 1.0080x over previous
"""Trainium2 Bass kernel for nn_MultiHeadAttn (B=4, NQ=NK=2048, D=1024, H=8).

Sharding: 8 cores = 4 batches x 2 query-halves. Each core owns 1024 query rows
of one batch; k/v projections for that batch are computed redundantly by the
two cores sharing it (cheap after key compaction + fp8).

Key compaction: the mask is host-visible and ~50% of keys are masked
(their attention weight is exactly 0), so the host gathers the unmasked
keys per batch and pads to KCAP (multiple of 256). This halves kproj,
vproj, logits, A*V, den and the exp volume.

Precision: the attention branch is strongly attenuated in the output
(softmax over ~1K near-uniform keys -> att is ~3% of the residual qp), so
it runs in fp8e4m3 with DoubleRow matmuls (2 fp8 contraction elems/cell):
k, v, Wk*16, Wv*16, vp*16 and exp(logits) are fp8. The residual path
(qproj, MLP, layernorms) runs in bf16 with f32 accumulation.

Per-core dataflow (activations feature-major "T layout" [feat, row]):
  qpT = Wq @ qT            (bf16)
  vp  = v @ Wv.T * 16      (fp8 DoubleRow, natural [key, feat] layout, fp8 out)
  per head: kpT_h = (Wk*16) @ kT   (fp8 DoubleRow, bf16 out at 16x scale)
  per head, per 512-row chunk, over KCAP/256 key-tile pairs:
      logitsT[kk,r] = kpT_h_tile.T @ qpT_h    (bf16 matmul, PSUM f32)
      expT = Exp(logitsT / 512)               (ACT, fp8 out; /512 = /16/32)
      attT += vp_pair.T @ expT                (fp8 DoubleRow accumulate)
      den  += mones.T @ expT                  (fp8, 32-row col-tiled blocks)
  x1T = qpT + attT * (1/(16*den))             (the /16 undoes the vp scale)
  out1 = LN(x1) via ones-matmul stats (sums over feature partitions)
  x2T = out1 + Relu(Wout @ out1T + bout)      (bf16 matmul, ACT bias+relu)
  outT = LN(x2)  -> DRAM [feat, row]; host transposes back.
"""

from contextlib import ExitStack

import numpy as np
import ml_dtypes

import concourse.mybir as mybir
import concourse.tile as tile
from concourse import bacc
from concourse.bass_utils import run_bass_kernel_spmd

B, NQ, NK, D, H = 4, 2048, 2048, 1024, 8
DH = D // H            # 128, head dim
P = 128                # partitions
RQ = NQ // 2           # 1024 query rows per core
EPS = 1e-5

F32 = mybir.dt.float32
BF16 = mybir.dt.bfloat16
FP8 = mybir.dt.float8e4
BFNP = ml_dtypes.bfloat16
F8NP = ml_dtypes.float8_e4m3

KT = D // P            # 8 contraction tiles over features
DT = D // P            # 8 output-feature tiles (also heads)
RC = RQ // 512         # 2 row chunks of 512
DR = mybir.MatmulPerfMode.DoubleRow
DRS = mybir.MatmulPerfMode.DoubleRowSwInterleave


def _swi_pairs(tiles):
    """Host prep for DoubleRowSwInterleave stationary operands.

    tiles: [T, P, M] — T contraction tiles of an [K=P, M] weight block.
    Returns [P, T//2, M, 2] where block (tp) holds, per partition, the
    interleaved reversed columns: raw[p, 2j+i] = tiles[2tp+i, p, M-1-j].
    """
    T, Pp, M = tiles.shape
    a = tiles.reshape(T // 2, 2, Pp, M)[:, :, :, ::-1]   # [tp, i, p, j]
    return np.ascontiguousarray(a.transpose(2, 0, 3, 1))  # [p, tp, j, i]


def build_nc(kcap, debug=False):
    """kcap: padded (compacted) key count, multiple of 256."""
    assert kcap % 256 == 0
    KKT = kcap // 128          # key tiles
    KPAIR = KKT // 2           # DoubleRow key-tile pairs
    # key chunks for the kproj output (N dim), each <= 512
    kchunks = []
    o = 0
    while o < kcap:
        n = min(512, kcap - o)
        kchunks.append((o, n))
        o += n

    nc = bacc.Bacc("TRN2", target_bir_lowering=False, debug=debug)

    qT = nc.declare_dram_parameter("qT", [D, RQ], BF16, isOutput=False)
    kT = nc.declare_dram_parameter("kT", [D, kcap], FP8, isOutput=False)
    # vT / wkT ship in DoubleRowSwInterleave layout: [P, T/2 pairs, M*2]
    vT = nc.declare_dram_parameter("vT", [P, (KT // 2) * kcap * 2], FP8, isOutput=False)
    wqT = nc.declare_dram_parameter("wqT", [D, D], BF16, isOutput=False)
    wkT = nc.declare_dram_parameter("wkT", [P, (KT // 2) * D * 2], FP8, isOutput=False)
    wvT = nc.declare_dram_parameter("wvT", [D, D], FP8, isOutput=False)
    woT = nc.declare_dram_parameter("woT", [D, D], BF16, isOutput=False)
    mones = nc.declare_dram_parameter("mones", [P, KKT * 32], FP8, isOutput=False)
    g1 = nc.declare_dram_parameter("g1", [P, DT], F32, isOutput=False)
    b1 = nc.declare_dram_parameter("b1", [P, DT], F32, isOutput=False)
    g2 = nc.declare_dram_parameter("g2", [P, DT], F32, isOutput=False)
    b2 = nc.declare_dram_parameter("b2", [P, DT], F32, isOutput=False)
    bo = nc.declare_dram_parameter("bo", [P, DT], F32, isOutput=False)
    outT = nc.declare_dram_parameter("outT", [D, RQ], F32, isOutput=True)

    Act = mybir.ActivationFunctionType

    with tile.TileContext(nc) as tc, ExitStack() as ctx:
        consts = ctx.enter_context(tc.tile_pool(name="consts", bufs=1))
        pool_qp = ctx.enter_context(tc.tile_pool(name="pool_qp", bufs=1))

        onesn = consts.tile([P, P], BF16)
        nc.vector.memset(onesn, 1.0 / D)
        eps_sb = consts.tile([P, 1], F32)
        nc.vector.memset(eps_sb, EPS)
        # den-broadcast lhsT: the partition sum over dsb yields 32*den (4
        # blocks x 32 replicas); with weight 16/32 the result is 16*den, whose
        # reciprocal also undoes the vp*16 scale when multiplied into att.
        ones32 = consts.tile([P, P], BF16)
        nc.vector.memset(ones32, 16.0 / 32.0)
        mones_sb = consts.tile([P, KKT, 32], FP8)
        nc.sync.dma_start(out=mones_sb, in_=mones[:, :])
        g1_sb = consts.tile([P, DT], F32)
        nc.sync.dma_start(out=g1_sb, in_=g1[:, :])
        b1_sb = consts.tile([P, DT], F32)
        nc.sync.dma_start(out=b1_sb, in_=b1[:, :])
        g2_sb = consts.tile([P, DT], F32)
        nc.sync.dma_start(out=g2_sb, in_=g2[:, :])
        b2_sb = consts.tile([P, DT], F32)
        nc.sync.dma_start(out=b2_sb, in_=b2[:, :])
        bo_sb = consts.tile([P, DT], F32)
        nc.sync.dma_start(out=bo_sb, in_=bo[:, :])

        # persistent activations
        qpT_sb = pool_qp.tile([P, DT, RQ], F32)      # qp.T; becomes x1T then x2T
        xbf_sb = pool_qp.tile([P, DT, RQ], BF16)     # bf16 shadow (qp, then x1, x2)

        with (
            tc.tile_pool(name="pool_attn", bufs=1) as pool_attn,
            tc.tile_pool(name="pool_ain", bufs=1) as ain,
        ):
            kpT_sb = pool_attn.tile([P, H, kcap], BF16)  # per-head [dh, key], 16x
            vp_sb = pool_attn.tile([P, KKT, D], FP8)     # per key-tile [key, feat], 16x
            # ------------- Phase A: q and v projections ----------
            # k is projected per-head inside the attention section so PE work
            # interleaves with the ACT-bound exp stream.
            with tc.tile_pool(name="a_ps", bufs=3, space="PSUM") as a_ps:
                # per-kt tiles so the first matmul only waits on its own slice
                wqA_t = [ain.tile([P, 512], BF16, tag=f"wqa{t}", name=f"wqA{t}")
                         for t in range(KT)]
                qT_t = [ain.tile([P, RQ], BF16, tag=f"qt{t}", name=f"qTs{t}")
                        for t in range(KT)]
                for t in range(KT):
                    nc.sync.dma_start(out=wqA_t[t], in_=wqT[t * P:(t + 1) * P, 0:512])
                    nc.sync.dma_start(out=qT_t[t][:, 0:512], in_=qT[t * P:(t + 1) * P, 0:512])
                    nc.sync.dma_start(out=qT_t[t][:, 512:1024], in_=qT[t * P:(t + 1) * P, 512:1024])
                wqB_t = [ain.tile([P, 512], BF16, tag=f"wqa{t}", name=f"wqB{t}")
                         for t in range(KT)]
                for t in range(KT):
                    nc.sync.dma_start(out=wqB_t[t], in_=wqT[t * P:(t + 1) * P, 512:1024])
                vT_sb = ain.tile([P, KT // 2, kcap * 2], FP8, tag="vv")
                for t in range(KT // 2):
                    nc.sync.dma_start(
                        out=vT_sb[:, t, :],
                        in_=vT[:, t * kcap * 2:(t + 1) * kcap * 2])

                def q_proj(w_t, dt0):
                    for dt_ in range(dt0, dt0 + 4):
                        for c in range(RC):
                            ps = a_ps.tile([P, 512], F32, tag="aps")
                            for kt in range(KT):
                                nc.tensor.matmul(
                                    ps,
                                    w_t[kt][:, (dt_ - dt0) * P:(dt_ - dt0 + 1) * P],
                                    qT_t[kt][:, c * 512:(c + 1) * 512],
                                    start=(kt == 0), stop=(kt == KT - 1),
                                )
                            nc.vector.tensor_copy(qpT_sb[:, dt_, c * 512:(c + 1) * 512], ps)
                            nc.vector.tensor_copy(xbf_sb[:, dt_, c * 512:(c + 1) * 512], ps)

                q_proj(wqA_t, 0)
                q_proj(wqB_t, 4)

                # v projection (fp8 DoubleRow): vp[kk, dout] = (v @ Wv.T) * 16
                # padded key rows are exactly zero (zero input columns).
                wvT_sb = ain.tile([P, KT, D], FP8, tag="w")
                for t in range(KT):
                    nc.sync.dma_start(out=wvT_sb[:, t, :], in_=wvT[t * P:(t + 1) * P, :])
                kT_sb = ain.tile([P, KT, kcap], FP8, tag="qk")
                for t in range(KT):
                    nc.sync.dma_start(out=kT_sb[:, t, :], in_=kT[t * P:(t + 1) * P, :])
                for kkt in range(KKT):
                    jb = (KKT - 1 - kkt) * 256
                    for c in range(D // 512):
                        ps = a_ps.tile([P, 512], F32, tag="aps")
                        for tp in range(KT // 2):
                            nc.tensor.matmul(
                                ps,
                                vT_sb[:, tp, jb:jb + 256],
                                wvT_sb[:, 2 * tp:2 * tp + 2, c * 512:(c + 1) * 512],
                                start=(tp == 0), stop=(tp == KT // 2 - 1),
                                perf_mode=DRS,
                            )
                        nc.vector.tensor_copy(vp_sb[:, kkt, c * 512:(c + 1) * 512], ps)

            wkT_sb = ain.tile([P, KT // 2, D * 2], FP8, tag="wk")
            for t in range(KT // 2):
                nc.sync.dma_start(
                    out=wkT_sb[:, t, :],
                    in_=wkT[:, t * D * 2:(t + 1) * D * 2])

            # den col-tiled block bookkeeping: block q accumulates kkt==q (mod 4)
            den_last = {q: max(k for k in range(KKT) if k % 4 == q) for q in range(4)}

            # ------------- Phase B: k projection + attention, per head -------
            # All retained keys are unmasked (masked keys were compacted away on
            # the host); zero-padded tail keys are excluded via zeroed vp rows
            # and zeroed den lhsT (mones). Emission is software-pipelined two
            # key-tile pairs ahead, and each (h,c) iteration's drain chain
            # (den sum -> reciprocal -> normalize -> residual) is deferred into
            # the next iteration so the in-order PE stream never waits.
            with (
                tc.tile_pool(name="att_ps", bufs=1, space="PSUM") as att_psp,
                tc.tile_pool(name="den_ps", bufs=1, space="PSUM") as den_psp,
                tc.tile_pool(name="lg_ps", bufs=2, space="PSUM") as lg_psp,
                tc.tile_pool(name="kp_ps", bufs=2, space="PSUM") as kp_psp,
                tc.tile_pool(name="bsb", bufs=1) as bsb,
            ):
                pending = []    # deferred drain state: (h, rs, att_ps, den_ps)

                def emit_drain():
                    if not pending:
                        return
                    h, rs, att_ps, den_ps = pending.pop()
                    # den blocks -> bf16 SBUF -> (1/512)-matmul sum + broadcast
                    dsb = bsb.tile([P, 512], BF16, tag="dsb", bufs=1)
                    nc.vector.tensor_copy(dsb, den_ps)
                    dbc = den_psp.tile([P, 512], F32, tag="den")
                    nc.tensor.matmul(dbc, ones32, dsb, start=True, stop=True)
                    rec = bsb.tile([P, 512], F32, tag="rec", bufs=1)
                    nc.vector.reciprocal_approx_fast(rec, dbc)
                    nc.vector.tensor_mul(rec, att_ps, rec)  # in-place att/(16 den)
                    # x1 = qp + att  (in place over qpT)
                    nc.vector.tensor_add(qpT_sb[:, h, rs], qpT_sb[:, h, rs], rec)
                    nc.vector.tensor_copy(xbf_sb[:, h, rs], qpT_sb[:, h, rs])

                for h in range(H):
                    # k projection for this head: kpT[h, :] = (Wk*16) @ k.T
                    jb = (H - 1 - h) * 256
                    for (co, cn) in kchunks:
                        ps = kp_psp.tile([P, 512], F32, tag="kp")
                        for tp in range(KT // 2):
                            nc.tensor.matmul(
                                ps[:, 0:cn],
                                wkT_sb[:, tp, jb:jb + 256],
                                kT_sb[:, 2 * tp:2 * tp + 2, co:co + cn],
                                start=(tp == 0), stop=(tp == KT // 2 - 1),
                                perf_mode=DRS,
                            )
                        nc.vector.tensor_copy(kpT_sb[:, h, co:co + cn], ps[:, 0:cn])

                    for c in range(RC):
                        rs = slice(c * 512, (c + 1) * 512)
                        att_ps = att_psp.tile([P, 512], F32, tag="att")
                        den_ps = den_psp.tile([P, 512], F32, tag="den")
                        exs = [None] * KPAIR

                        def emit_lgexp(g):
                            lg_ps = lg_psp.tile([P, 2, 512], F32, tag="lg")
                            for j in range(2):
                                kkt = 2 * g + j
                                nc.tensor.matmul(
                                    lg_ps[:, j, :],
                                    kpT_sb[:, h, kkt * P:(kkt + 1) * P],
                                    xbf_sb[:, h, rs],
                                    start=True, stop=True,
                                )
                            ex = bsb.tile([P, 2, 512], FP8, tag="ex", bufs=KPAIR + 1)
                            # /512 = /16 (kp scale) /32 (sqrt(D))
                            nc.scalar.activation(ex, lg_ps, Act.Exp, scale=1.0 / 512.0)
                            exs[g] = ex

                        def emit_att(g):
                            nc.tensor.matmul(
                                att_ps,
                                vp_sb[:, 2 * g:2 * g + 2, h * DH:(h + 1) * DH],
                                exs[g],
                                start=(g == 0), stop=(g == KPAIR - 1),
                                perf_mode=DR,
                            )

                        def emit_den(k0, k1):
                            # den matmuls back-to-back so the 32-col tiles
                            # pack concurrently in the array
                            for kkt in range(k0, k1):
                                q = kkt % 4
                                nc.tensor.matmul(
                                    den_ps[32 * q:32 * (q + 1), :],
                                    mones_sb[:, kkt, :],
                                    exs[kkt // 2][:, kkt % 2, :],
                                    start=(kkt < 4),
                                    stop=(kkt == den_last[q]),
                                    tile_position=(0, 32 * q),
                                    skip_group_check=True,
                                )

                        emit_lgexp(0)
                        emit_lgexp(1)
                        emit_drain()  # previous iteration's normalize chain
                        den_done = 0
                        for g in range(2, KPAIR):
                            emit_lgexp(g)
                            emit_att(g - 2)
                            # den for 4 key tiles at a time, two ex-pairs back
                            if 2 * (g - 1) - den_done >= 4:
                                emit_den(den_done, den_done + 4)
                                den_done += 4
                        emit_att(KPAIR - 2)
                        emit_att(KPAIR - 1)
                        emit_den(den_done, KKT)
                        pending.append((h, rs, att_ps, den_ps))

                emit_drain()  # final iteration

        # ---------------- Phase C/D: LN1, MLP, LN2 ----------------
        with (
            tc.tile_pool(name="late", bufs=1) as late,
            tc.tile_pool(name="csb", bufs=1) as csb,
            tc.tile_pool(name="c_ps", bufs=2, space="PSUM") as c_ps,
        ):
            woT_sb = late.tile([P, KT, D], BF16)
            for t in range(KT):
                nc.sync.dma_start(out=woT_sb[:, t, :], in_=woT[t * P:(t + 1) * P, :])
            x1n_sb = late.tile([P, DT, RQ], BF16)

            def ln_stats_rc(src, mean_srcs, c):
                """stats for one row-chunk: returns (mean_ps, rsg) both
                [P,512], identical across partitions. mean_srcs: list of bf16
                [P,512] APs whose per-feature sum is the row vector."""
                rs = slice(c * 512, (c + 1) * 512)
                mean_ps = c_ps.tile([P, 512], F32, tag="mean")
                for i, ms in enumerate(mean_srcs):
                    nc.tensor.matmul(
                        mean_ps, onesn, ms,
                        start=(i == 0), stop=(i == len(mean_srcs) - 1),
                    )
                msq_ps = c_ps.tile([P, 512], F32, tag="msq")
                for kt in range(KT):
                    sq = csb.tile([P, 512], BF16, tag="sq", bufs=3)
                    nc.scalar.square(sq, src[:, kt, rs])
                    nc.tensor.matmul(
                        msq_ps, onesn, sq,
                        start=(kt == 0), stop=(kt == KT - 1),
                    )
                musq = csb.tile([P, 512], F32, tag="musq", bufs=2)
                nc.scalar.square(musq, mean_ps)
                var = csb.tile([P, 512], F32, tag="var", bufs=2)
                nc.vector.tensor_sub(var, msq_ps, musq)
                std = csb.tile([P, 512], F32, tag="std", bufs=2)
                nc.scalar.activation(std, var, Act.Sqrt, bias=eps_sb[:, :], scale=1.0)
                rsg = csb.tile([P, 512], F32, tag="rsg", bufs=2)
                nc.vector.reciprocal_approx_fast(rsg, std)
                return mean_ps, rsg

            def normalize(src, mean_ps, rsg, emit_out, rs):
                # (x - mean) * rsg per feature tile; sub/mul alternate between
                # DVE and GpSimd to split the element-wise load.
                for kt in range(DT):
                    eng = nc.vector
                    xc = csb.tile([P, 512], F32, tag="xc", bufs=4)
                    eng.tensor_sub(xc, src[:, kt, rs], mean_ps)
                    xh = csb.tile([P, 512], F32, tag="xh", bufs=4)
                    eng.tensor_mul(xh, xc, rsg)
                    emit_out(kt, xh)

            # LN1: x1n = LN(x1) * g1 + b1   (ACT affine, bf16 out)
            for c in range(RC):
                rs = slice(c * 512, (c + 1) * 512)
                mean_ps, rsg = ln_stats_rc(
                    qpT_sb, [xbf_sb[:, kt, rs] for kt in range(KT)], c)

                def ln1_out(kt, xh, rs=rs):
                    nc.scalar.activation(
                        x1n_sb[:, kt, rs], xh, Act.Identity,
                        bias=b1_sb[:, kt:kt + 1], scale=g1_sb[:, kt:kt + 1],
                    )
                normalize(qpT_sb, mean_ps, rsg, ln1_out, rs)

            # MLP: x2 = x1n + relu(Wout @ x1n.T + bout)  (x2 overwrites qpT),
            # then LN2 of that row-chunk. The LN2 mean is accumulated from the
            # x1n and relu bf16 tiles directly (no x2 bf16 shadow needed).
            for c in range(RC):
                rs = slice(c * 512, (c + 1) * 512)
                rls = []
                for dt_ in range(DT):
                    z_ps = c_ps.tile([P, 512], F32, tag="z")
                    for kt in range(KT):
                        nc.tensor.matmul(
                            z_ps,
                            woT_sb[:, kt, dt_ * P:(dt_ + 1) * P],
                            x1n_sb[:, kt, rs],
                            start=(kt == 0), stop=(kt == KT - 1),
                        )
                    rl = csb.tile([P, 512], BF16, tag="rl", bufs=DT)
                    nc.scalar.activation(
                        rl, z_ps, Act.Relu,
                        bias=bo_sb[:, dt_:dt_ + 1], scale=1.0,
                    )
                    nc.vector.tensor_add(qpT_sb[:, dt_, rs], x1n_sb[:, dt_, rs], rl)
                    rls.append(rl)

                mean_ps, rsg = ln_stats_rc(
                    qpT_sb, [x1n_sb[:, kt, rs] for kt in range(KT)] + rls, c)

                def ln2_out(kt, xh, rs=rs):
                    ot = csb.tile([P, 512], F32, tag="ot", bufs=3)
                    nc.scalar.activation(
                        ot, xh, Act.Identity,
                        bias=b2_sb[:, kt:kt + 1], scale=g2_sb[:, kt:kt + 1],
                    )
                    nc.sync.dma_start(out=outT[kt * P:(kt + 1) * P, rs], in_=ot)
                normalize(qpT_sb, mean_ps, rsg, ln2_out, rs)

    nc.compile()
    return nc


_NC_CACHE = {}


def get_nc(kcap=1280, debug=False):
    key = (kcap, debug)
    if key not in _NC_CACHE:
        _NC_CACHE[key] = build_nc(kcap, debug=debug)
    return _NC_CACHE[key]


def choose_kcap(mask):
    nkeep = int((~np.asarray(mask)).sum(axis=1).max())
    return max(256, -(-nkeep // 256) * 256)


def shard_inputs(q, k, v, mask, Wq, Wk, Wv, Wout, bout, g1, b1, g2, b2,
                 kcap=None):
    q = np.asarray(q, dtype=np.float32)
    k = np.asarray(k, dtype=np.float32)
    v = np.asarray(v, dtype=np.float32)
    mask = np.asarray(mask)
    if kcap is None:
        kcap = choose_kcap(mask)
    KKT = kcap // 128
    bfc = lambda a: np.ascontiguousarray(np.asarray(a, dtype=np.float32)).astype(BFNP)
    f8c = lambda a: np.ascontiguousarray(np.asarray(a, dtype=np.float32)).astype(F8NP)
    vec = lambda a: np.ascontiguousarray(
        np.asarray(a, dtype=np.float32).reshape(DT, P).T
    )

    wk_swi = _swi_pairs((np.asarray(Wk, np.float32).T * 16.0).reshape(KT, P, D))
    shared = {
        "wqT": bfc(np.asarray(Wq, np.float32).T),
        "wkT": f8c(wk_swi.reshape(P, -1)),
        "wvT": f8c(np.asarray(Wv, np.float32).T * 16.0),
        "woT": bfc(np.asarray(Wout, np.float32).T),
        "g1": vec(g1), "b1": vec(b1), "g2": vec(g2), "b2": vec(b2),
        "bo": vec(bout),
    }
    in_maps = []
    for bi in range(B):
        keep = np.where(~mask[bi])[0]
        nk = len(keep)
        kc = np.zeros((D, kcap), np.float32)
        vc = np.zeros((D, kcap), np.float32)
        kc[:, :nk] = k[bi][keep].T
        vc[:, :nk] = v[bi][keep].T
        vc = _swi_pairs(vc.reshape(KT, P, kcap)).reshape(P, -1)
        mo = np.zeros((kcap, 32), np.float32)   # [key, 32] -> [P, KKT*32]
        mo[:nk] = 1.0
        mo = mo.reshape(KKT, P, 32).transpose(1, 0, 2).reshape(P, KKT * 32)
        per_batch = {
            "kT": f8c(kc),
            "vT": f8c(vc),
            "mones": f8c(mo),
            **shared,
        }
        for half in range(2):
            rows = slice(half * RQ, (half + 1) * RQ)
            in_maps.append({
                "qT": bfc(q[bi, rows].T),
                **per_batch,
            })
    return in_maps


def assemble_output(results):
    out = np.empty((B, NQ, D), dtype=np.float32)
    for c in range(8):
        bi, half = divmod(c, 2)
        rows = slice(half * RQ, (half + 1) * RQ)
        out[bi, rows, :] = results[c]["outT"].T
    return out


def kernel(**inputs):
    kcap = choose_kcap(inputs["mask"])
    nc = get_nc(kcap)
    in_maps = shard_inputs(**inputs, kcap=kcap)
    res = run_bass_kernel_spmd(nc, in_maps, core_ids=list(range(8)))
    return assemble_output(res.results)
